# revision 40
# baseline (speedup 1.0000x reference)
"""Trainium2 Bass kernel for nn_BertCosAttention (B=8, S=2048, HID=1024, H=16, DH=64).

Sharding: data-parallel over batch, 1 batch per NeuronCore, 8 cores, no
collectives.  All FLOPs on device:
  k,v = hT^T @ W      fp8e4 DoubleRow matmuls (2 contraction chunks/pass)
  q^T = Wq^T @ hT     bf16 (fp8 Q fails the accuracy budget)
  sumsq_q = E^T @ q^2 fp8 DoubleRow E-matmul -> rs_q = 1/sqrt(vs2*ss + eps)
  kv[h] = k_n[h]^T @ v[h]   bf16, accumulated over s-chunks
  ctx   = q^T^T @ kv_blockdiag, rows scaled by rs_q (DVE)
Host pre-scales the fp8 operands into e4m3 normal range (h*16, W*256); the
resulting 4096x on the k/v psum vanishes in the k l2-norm and is folded into
rs_q via vs2 = (count*4096)^2/16 (the /16 absorbs the Square-activation
scale=4 used when quantizing q^2 to fp8).
Phase order: K/V+kv first, then per 512-seq chunk: Q proj -> rs chain -> ctx
-> store, so output DMA drains during compute instead of in a tail.
"""

import numpy as np

import concourse.bacc as bacc
import concourse.mybir as mybir
from concourse import tile
from concourse.bass_utils import run_bass_kernel_spmd

B, S, HID = 8, 2048, 1024
H, DH = 16, 64
P = 128                       # partitions
NG = HID // P                 # 8 column/row groups of 128
NSC = S // P                  # 16 seq chunks of 128
NSJ = S // 512                # 4 seq chunks of 512
KCH = HID // P                # 8 contraction chunks
KK2 = KCH // 2                # 4 double-contraction chunks (fp8 DoubleRow)
EPS = 1e-24
HSCALE = 16.0                 # h -> fp8 prescale
WSCALE = 256.0                # W -> fp8 prescale
QSQ_SCALE = 4.0               # Square-activation input scale for q^2 -> fp8

F32 = mybir.dt.float32
BF16 = mybir.dt.bfloat16
F8 = mybir.dt.float8e4
AF = mybir.ActivationFunctionType
DR = mybir.MatmulPerfMode.DoubleRow


def build(has_kv_bias: bool):
    nc = bacc.Bacc("TRN2", target_bir_lowering=False, debug=False, num_devices=8)

    # All DRAM inputs are pre-arranged host-side as exact SBUF images
    # [128, free] so each load is one big call with contiguous per-partition
    # descriptors (>=4KB) instead of 128 small row descriptors.
    ht_d = nc.dram_tensor("ht", [P, NG * S], BF16, kind="ExternalInput")
    ht8_d = nc.dram_tensor("ht8", [P, KCH * S], F8, kind="ExternalInput")
    wq_d = nc.dram_tensor("wq", [P, KCH * HID], BF16, kind="ExternalInput")
    w8k_d = nc.dram_tensor("w8k", [P, KCH * HID], F8, kind="ExternalInput")
    w8v_d = nc.dram_tensor("w8v", [P, KCH * HID], F8, kind="ExternalInput")
    e8_d = nc.dram_tensor("e8", [P, KCH * H], F8, kind="ExternalInput")
    i16_d = nc.dram_tensor("i16", [16, 16], F32, kind="ExternalInput")
    vs2_d = nc.dram_tensor("vs2", [P, 1], F32, kind="ExternalInput")
    bqt_d = nc.dram_tensor("bqt", [P, NG], F32, kind="ExternalInput")
    if has_kv_bias:
        bk_d = nc.dram_tensor("bk", [P, HID], F32, kind="ExternalInput")
        bv_d = nc.dram_tensor("bv", [P, HID], F32, kind="ExternalInput")
    out_d = nc.dram_tensor("out", [S, HID], F32, kind="ExternalOutput")

    with tile.TileContext(nc) as tc:
        with (
            tc.tile_pool(name="persist", bufs=1) as pp,
            tc.tile_pool(name="work", bufs=3) as wp,
            tc.tile_pool(name="outp", bufs=3) as op,
        ):
            # ---- constants (gpsimd: all are Q-phase-only, keep the sync
            # HWDGE lane free for the critical K/V loads) ------------------
            e8_sb = pp.tile([P, KCH * H], F8, tag="e8")
            i16_sb = pp.tile([16, 16], F32, tag="i16")
            vs2_sb = pp.tile([P, 1], F32, tag="vs2")
            bqt_sb = pp.tile([P, NG], F32, tag="bqt")
            eps_sb = pp.tile([P, 1], F32, tag="eps")
            nc.vector.memset(eps_sb[:], EPS)
            if has_kv_bias:
                bk_sb = pp.tile([P, HID], F32, tag="bk")
                nc.gpsimd.dma_start(bk_sb[:], bk_d[:])
                bv_sb = pp.tile([P, HID], F32, tag="bv")
                nc.gpsimd.dma_start(bv_sb[:], bv_d[:])

            # ---- big SBUF tensors + staged loads ------------------------
            w8k_sb = pp.tile([P, KCH * HID], F8, tag="w8k")
            w8v_sb = pp.tile([P, KCH * HID], F8, tag="w8v")
            ht8_sb = pp.tile([P, KCH * S], F8, tag="ht8")
            wq_sb = pp.tile([P, KCH * HID], BF16, tag="wq")
            hta = pp.tile([P, NG * S], BF16, tag="hta")
            # SBUF layouts: w8 [p, (nj, r, o512)]; ht8 [p, (b, r, s512)];
            # wq [p, (kk, o)]; hta [p, (b, g, s512)]  (b = 512-seq block)
            w8k4 = w8k_sb[:].rearrange("p (nj r o) -> p nj r o", r=KCH, o=512)
            w8v4 = w8v_sb[:].rearrange("p (nj r o) -> p nj r o", r=KCH, o=512)
            ht84 = ht8_sb[:].rearrange("p (b r s) -> p b r s", r=KCH, s=512)
            wq3 = wq_sb[:].rearrange("p (r o) -> p r o", o=HID)
            ht4 = hta[:].rearrange("p (b g s) -> p b g s", g=NG, s=512)
            e83 = e8_sb[:].rearrange("p (r h) -> p r h", h=H)

            # Loads split over three ordered initiator lanes so the per-call
            # completion latencies of the critical K/V tensors overlap;
            # within each lane, priority order (FIFO per hw queue).
            HKB = KCH * 512
            nc.sync.dma_start(ht8_sb[:, 0:HKB], ht8_d[:, 0:HKB])
            nc.sync.dma_start(w8k_sb[:, 0:HKB], w8k_d[:, 0:HKB])
            nc.sync.dma_start(w8k_sb[:, HKB:], w8k_d[:, HKB:])
            nc.sync.dma_start(ht8_sb[:, HKB:], ht8_d[:, HKB:])
            nc.sync.dma_start(hta[:], ht_d[:])
            nc.scalar.dma_start(w8v_sb[:], w8v_d[:])
            nc.scalar.dma_start(wq_sb[:], wq_d[:])
            nc.gpsimd.dma_start(e8_sb[:], e8_d[:])
            nc.gpsimd.dma_start(i16_sb[:], i16_d[:])
            nc.gpsimd.dma_start(vs2_sb[:], vs2_d[:])
            nc.gpsimd.dma_start(bqt_sb[:], bqt_d[:])
            bqt4_sb = pp.tile([P, NG], F32, tag="bqt4")
            nc.vector.tensor_scalar_mul(bqt4_sb[:], bqt_sb[:], QSQ_SCALE)

            # ---- persistent results -------------------------------------
            qt = [pp.tile([P, S], BF16, name=f"qt{g}", tag=f"qt{g}")
                  for g in range(NG)]
            sst = pp.tile([16, S], F32, tag="sst")
            rs_all = pp.tile([P, NSC * H], F32, tag="rsall")
            rs_q = [rs_all[:, sc * H : (sc + 1) * H] for sc in range(NSC)]
            kv_sb = pp.tile([P, H * DH], BF16, tag="kvsb")
            nc.vector.memset(kv_sb[:], 0.0)

            # ---- K/V projections (fp8 DoubleRow) + kv accumulation ------
            with (
                tc.tile_pool(name="kprj", bufs=4, space="PSUM") as kvpk,
                tc.tile_pool(name="vprj", bufs=3, space="PSUM") as kvpv,
                tc.tile_pool(name="kvacc", bufs=1, space="PSUM") as kva,
            ):
                # even heads accumulate on partitions 0-63, odd on 64-127:
                # col-group tiling lets even/odd matmuls run concurrently
                kv_ps = kva.tile([P, H * 32], F32, name="kvacc", tag="kvacc")
                kn_t = {}

                def _emit_kv(esc):
                    ekn, evs = kn_t.pop(esc)
                    for hh in range(H):
                        po = (hh % 2) * DH
                        co = (hh // 2) * DH
                        nc.tensor.matmul(
                            kv_ps[po : po + DH, co : co + DH],
                            ekn[:, hh * DH : (hh + 1) * DH],
                            evs[:, hh * DH : (hh + 1) * DH],
                            start=(esc == 0 and hh < 2),
                            stop=(esc == NSC - 1 and hh >= H - 2),
                        )

                for sc in range(NSC):
                    k_ps = [kvpk.tile([P, 512], F32, name=f"kp{sc}_{i}", tag="kp")
                            for i in range(2)]
                    v_ps = [kvpv.tile([P, 512], F32, name=f"vp{sc}_{i}", tag="vp")
                            for i in range(2)]
                    scb, sco = sc // 4, (sc % 4) * P
                    for kk2 in range(KK2):
                        lhs = ht84[:, scb, 2 * kk2 : 2 * kk2 + 2,
                                   sco : sco + P]
                        for nj in range(2):
                            nc.tensor.matmul(
                                k_ps[nj][:],
                                lhs,
                                w8k4[:, nj, 2 * kk2 : 2 * kk2 + 2, :],
                                start=(kk2 == 0),
                                stop=(kk2 == KK2 - 1),
                                perf_mode=DR,
                            )
                        for nj in range(2):
                            nc.tensor.matmul(
                                v_ps[nj][:],
                                lhs,
                                w8v4[:, nj, 2 * kk2 : 2 * kk2 + 2, :],
                                start=(kk2 == 0),
                                stop=(kk2 == KK2 - 1),
                                perf_mode=DR,
                            )
                    if has_kv_bias:
                        for nj in range(2):
                            nc.vector.tensor_add(
                                k_ps[nj][:], k_ps[nj][:],
                                bk_sb[:, nj * 512 : (nj + 1) * 512],
                            )
                            nc.vector.tensor_add(
                                v_ps[nj][:], v_ps[nj][:],
                                bv_sb[:, nj * 512 : (nj + 1) * 512],
                            )

                    # row sumsq of k per head -> rs_k -> k_n; v -> bf16
                    # (chain split per 512-col half for earlier pipelining)
                    k_n = wp.tile([P, HID], BF16, name=f"kn{sc}", tag="kn",
                                  bufs=6)
                    v_sb = wp.tile([P, HID], BF16, name=f"vsb{sc}", tag="vsb",
                                  bufs=6)
                    for nj in range(2):
                        sl = slice(nj * 512, (nj + 1) * 512)
                        hsl = slice(nj * 8, (nj + 1) * 8)
                        ksq = wp.tile([P, 512], BF16, tag="ksq")
                        ssk = wp.tile([P, 8], F32, tag="ssk")
                        sqk = wp.tile([P, 8], F32, tag="sqk")
                        rsk = wp.tile([P, 8], F32, tag="rsk")
                        nc.scalar.activation(ksq[:], k_ps[nj][:], AF.Square)
                        if nj == 0:
                            nc.scalar.copy(v_sb[:, sl], v_ps[nj][:])
                        else:
                            nc.vector.tensor_copy(v_sb[:, sl], v_ps[nj][:])
                        nc.vector.tensor_reduce(
                            ssk[:],
                            ksq[:].rearrange("p (h d) -> p h d", d=DH),
                            axis=mybir.AxisListType.X,
                            op=mybir.AluOpType.add,
                        )
                        nc.scalar.activation(
                            sqk[:], ssk[:], AF.Sqrt, bias=eps_sb[:, 0:1]
                        )
                        nc.vector.reciprocal(rsk[:], sqk[:])
                        nc.vector.tensor_mul(
                            k_n[:].rearrange("p (h d) -> p h d", d=DH)[
                                :, hsl, :
                            ],
                            k_ps[nj][:].rearrange("p (h d) -> p h d", d=DH),
                            rsk[:, :, None].broadcast_to([P, 8, DH]),
                        )
                    kn_t[sc] = (k_n, v_sb)
                    # kv accumulation, deferred 2 chunks so the norm chain
                    # overlaps kv matmuls instead of stalling the PE
                    if sc >= 2:
                        _emit_kv(sc - 2)
                for sc in range(NSC - 2, NSC):
                    _emit_kv(sc)
                kvv = kv_sb[:].rearrange("p (pp two d) -> p pp two d",
                                          two=2, d=DH)
                kvp3 = kv_ps[:].rearrange("p (pp d) -> p pp d", d=DH)
                for hp2 in range(2):
                    pps = slice(hp2 * 4, (hp2 + 1) * 4)
                    nc.vector.tensor_copy(kvv[0:DH, pps, 0, :],
                                          kvp3[0:DH, pps, :])
                    nc.vector.tensor_copy(kvv[DH:P, pps, 1, :],
                                          kvp3[DH:P, pps, :])

            # ---- Q proj + sumsq + rs chain + ctx + store, per 512 seq ---
            with (
                tc.tile_pool(name="qpsum", bufs=2, space="PSUM") as qps,
                tc.tile_pool(name="sspsum", bufs=1, space="PSUM") as ssps,
                tc.tile_pool(name="rspsum", bufs=1, space="PSUM") as rsps,
                tc.tile_pool(name="ctxpsum", bufs=2, space="PSUM") as cps,
            ):
                def _emit_ctx_sc(sc):
                    # ctx for one seq chunk; kv_sb is block-diagonal per
                    # head pair, one K=128 matmul covers both heads
                    if True:
                        c_ps = cps.tile([P, HID], F32, tag="cp")
                        for pair in range(8):
                            nc.tensor.matmul(
                                c_ps[:, pair * P : (pair + 1) * P],
                                qt[pair][:, sc * P : (sc + 1) * P],
                                kv_sb[:, pair * P : (pair + 1) * P],
                                start=True,
                                stop=True,
                            )
                        out_t = op.tile([P, HID], F32, tag="outt")
                        nc.vector.tensor_mul(
                            out_t[:].rearrange("p (h d) -> p h d", d=DH),
                            c_ps[:].rearrange("p (h d) -> p h d", d=DH),
                            rs_all[:, sc * H : (sc + 1) * H][
                                :, :, None
                            ].broadcast_to([P, H, DH]),
                        )
                        nc.sync.dma_start(
                            out_d[sc * P : (sc + 1) * P, :], out_t[:]
                        )

                for j in range(NSJ):
                    jsl = slice(j * 512, (j + 1) * 512)
                    ss_ps = ssps.tile([16, 512], F32, tag="ssp")
                    qsq8 = None
                    for g in range(NG):
                        q_ps = qps.tile([P, 512], F32, tag="qp")
                        for kk in range(KCH):
                            nc.tensor.matmul(
                                q_ps[:],
                                wq3[:, kk, g * P : (g + 1) * P],
                                ht4[:, j, kk, :],
                                start=(kk == 0),
                                stop=(kk == KCH - 1),
                            )
                        # psum -> sbuf bf16 with per-partition bias add
                        nc.scalar.activation(
                            qt[g][:, jsl], q_ps[:], AF.Identity,
                            bias=bqt_sb[:, g : g + 1],
                        )
                        if g % 2 == 0:
                            qsq8 = wp.tile([P, 1024], F8, tag="qsq")
                        qsq83 = qsq8[:].rearrange("p (two f) -> p two f", f=512)
                        # (4q + 4bq)^2 = 16 q_biased^2 -> fp8, straight from
                        # psum so it doesn't wait on the Identity copy
                        nc.scalar.activation(
                            qsq83[:, g % 2, :], q_ps[:], AF.Square,
                            scale=QSQ_SCALE, bias=bqt4_sb[:, g : g + 1],
                        )
                        if g % 2 == 1:
                            gg = g // 2
                            nc.tensor.matmul(
                                ss_ps[:],
                                e83[:, 2 * gg : 2 * gg + 2, :],
                                qsq83[:, 0:2, :],
                                start=(gg == 0),
                                stop=(gg == KK2 - 1),
                                perf_mode=DR,
                            )
                            # spread the previous j's ctx through this
                            # j's projections: its DVE scales then overlap
                            # PE matmuls instead of bunching at the end
                            if j > 0:
                                _emit_ctx_sc(4 * (j - 1) + gg)
                    nc.vector.tensor_copy(sst[:, jsl], ss_ps[:])
                    # 4 transposes into one psum bank, then a single
                    # sqrt + reciprocal covering all 4 seq chunks of j
                    rs_ps = rsps.tile([P, 4 * H], F32, tag="rsp")
                    for i in range(4):
                        sc = 4 * j + i
                        nc.tensor.transpose(
                            rs_ps[:, i * H : (i + 1) * H],
                            sst[:, sc * P : (sc + 1) * P], i16_sb[:]
                        )
                    sq = wp.tile([P, 4 * H], F32, tag="sqq")
                    nc.scalar.activation(
                        sq[:], rs_ps[:], AF.Sqrt,
                        bias=eps_sb[:, 0:1], scale=vs2_sb[:, 0:1],
                    )
                    nc.vector.reciprocal(
                        rs_all[:, 4 * j * H : (4 * j + 4) * H], sq[:]
                    )
                for sc in range(4 * (NSJ - 1), 4 * NSJ):
                    _emit_ctx_sc(sc)

    nc.compile()
    return nc


_CACHE = {}


def _get_nc(has_kv_bias: bool):
    if has_kv_bias not in _CACHE:
        _CACHE[has_kv_bias] = build(has_kv_bias)
    return _CACHE[has_kv_bias]


def _prep_inputs(hidden_states, attention_mask, Wq, bq, Wk, bk, Wv, bv):
    """Host-side shard prep. Returns (in_maps, has_kv_bias)."""
    hs = np.asarray(hidden_states, dtype=np.float32)
    am = np.asarray(attention_mask)
    m = (am == 0).astype(np.float32).reshape(B, S)      # [B, S] valid mask
    counts = m.sum(axis=1)                               # [B]
    if not np.all(m == 1.0):
        hs = hs * m[:, :, None]                          # exact when biases==0

    wq = np.asarray(Wq, dtype=np.float32)
    wk = np.asarray(Wk, dtype=np.float32)
    wv = np.asarray(Wv, dtype=np.float32)
    bq_ = np.asarray(bq, dtype=np.float32)
    bk_ = np.asarray(bk, dtype=np.float32)
    bv_ = np.asarray(bv, dtype=np.float32)
    has_kv_bias = bool(np.any(bk_ != 0) or np.any(bv_ != 0))

    import ml_dtypes

    F8NP = ml_dtypes.float8_e4m3

    def w_img(w, dt):
        # [in, out] -> SBUF image [p, (nj, r, o512)]
        return np.ascontiguousarray(
            w.reshape(KCH, P, 2, 512).transpose(1, 2, 0, 3).reshape(P, -1)
        ).astype(dt)

    def h_img(ht, dt):
        # hT [hid, S] -> SBUF image [p, (b, r, s512)]
        return np.ascontiguousarray(
            ht.reshape(KCH, P, NSJ, 512).transpose(1, 2, 0, 3).reshape(P, -1)
        ).astype(dt)

    wq16 = np.ascontiguousarray(
        wq.reshape(KCH, P, HID).transpose(1, 0, 2).reshape(P, -1)
    ).astype(ml_dtypes.bfloat16)
    w8k = w_img(wk * WSCALE, F8NP)
    w8v = w_img(wv * WSCALE, F8NP)

    # e8[p, g*H + h] = 1 if hid index g*128+p belongs to head h
    o = np.arange(HID)
    e_full = (o[:, None] // DH == np.arange(H)[None, :]).astype(np.float32)
    e8 = np.ascontiguousarray(
        e_full.reshape(NG, P, H).transpose(1, 0, 2).reshape(P, NG * H)
    ).astype(F8NP)
    i16 = np.eye(16, dtype=np.float32)
    bqt = np.ascontiguousarray(bq_.reshape(NG, P).T)     # [128, 8]

    # k/v psum carries HSCALE*WSCALE = 4096x; ss carries 16x (QSQ_SCALE^2)
    pscale = HSCALE * WSCALE

    in_maps = []
    for b in range(B):
        htb = hs[b].T
        im = {
            "ht": h_img(htb, ml_dtypes.bfloat16),
            "ht8": h_img(htb * HSCALE, F8NP),
            "wq": wq16, "w8k": w8k, "w8v": w8v,
            "e8": e8, "i16": i16,
            "vs2": np.full(
                (P, 1),
                (np.float32(counts[b]) * pscale) ** 2 / QSQ_SCALE**2,
                np.float32,
            ),
            "bqt": bqt,
        }
        if has_kv_bias:
            im["bk"] = np.broadcast_to(bk_ * pscale, (P, HID)).copy()
            im["bv"] = np.broadcast_to(bv_ * pscale, (P, HID)).copy()
        in_maps.append(im)
    return in_maps, has_kv_bias


def run(inputs: dict, trace: bool = False, debug: bool = False):
    in_maps, has_kv_bias = _prep_inputs(**inputs)
    nc = _get_nc(has_kv_bias)
    res = run_bass_kernel_spmd(nc, in_maps, list(range(B)), trace=trace)
    out = np.stack([res.results[i]["out"] for i in range(B)]).astype(np.float32)
    return out, res


def kernel(**inputs) -> np.ndarray:
    out, _ = run(inputs)
    return out


# revision 42
# speedup vs baseline: 1.1454x; 1.1454x over previous
"""Trainium2 Bass kernel for nn_BertCosAttention (B=8, S=2048, HID=1024, H=16, DH=64).

Sharding: data-parallel over batch, 1 batch per NeuronCore, 8 cores, no
collectives.  All FLOPs on device:
  k,v = hT^T @ W      fp8e4 DoubleRow matmuls (2 contraction chunks/pass)
  q^T = Wq^T @ hT     bf16 (fp8 Q fails the accuracy budget)
  sumsq_q = E^T @ q^2 fp8 DoubleRow E-matmul -> rs_q = 1/sqrt(vs2*ss + eps)
  kv[h] = k_n[h]^T @ v[h]   bf16, accumulated over s-chunks
  ctx   = q^T^T @ kv_blockdiag, rows scaled by rs_q (DVE)
Host pre-scales the fp8 operands into e4m3 normal range (h*16, W*256); the
resulting 4096x on the k/v psum vanishes in the k l2-norm and is folded into
rs_q via vs2 = (count*4096)^2/16 (the /16 absorbs the Square-activation
scale=4 used when quantizing q^2 to fp8).
Phase order: K/V+kv first, then per 512-seq chunk: Q proj -> rs chain -> ctx
-> store, so output DMA drains during compute instead of in a tail.
"""

import numpy as np

import concourse.bacc as bacc
import concourse.mybir as mybir
from concourse import tile
from concourse.bass_utils import run_bass_kernel_spmd

B, S, HID = 8, 2048, 1024
H, DH = 16, 64
P = 128                       # partitions
NG = HID // P                 # 8 column/row groups of 128
NSC = S // P                  # 16 seq chunks of 128
NSJ = S // 512                # 4 seq chunks of 512
KCH = HID // P                # 8 contraction chunks
KK2 = KCH // 2                # 4 double-contraction chunks (fp8 DoubleRow)
EPS = 1e-24
HSCALE = 16.0                 # h -> fp8 prescale
WSCALE = 256.0                # W -> fp8 prescale
QSQ_SCALE = 4.0               # Square-activation input scale for q^2 -> fp8

F32 = mybir.dt.float32
BF16 = mybir.dt.bfloat16
F8 = mybir.dt.float8e4
AF = mybir.ActivationFunctionType
DR = mybir.MatmulPerfMode.DoubleRow


def build(has_kv_bias: bool):
    nc = bacc.Bacc("TRN2", target_bir_lowering=False, debug=False, num_devices=8)

    # All DRAM inputs are pre-arranged host-side as exact SBUF images
    # [128, free] so each load is one big call with contiguous per-partition
    # descriptors (>=4KB) instead of 128 small row descriptors.
    ht_d = nc.dram_tensor("ht", [P, NG * S], BF16, kind="ExternalInput")
    ht8_d = nc.dram_tensor("ht8", [P, KCH * S], F8, kind="ExternalInput")
    wq_d = nc.dram_tensor("wq", [P, KCH * HID], BF16, kind="ExternalInput")
    w8k_d = nc.dram_tensor("w8k", [P, KCH * HID], F8, kind="ExternalInput")
    w8v_d = nc.dram_tensor("w8v", [P, KCH * HID], F8, kind="ExternalInput")
    e8_d = nc.dram_tensor("e8", [P, KCH * H], F8, kind="ExternalInput")
    i16_d = nc.dram_tensor("i16", [16, 16], F32, kind="ExternalInput")
    vs2_d = nc.dram_tensor("vs2", [P, 1], F32, kind="ExternalInput")
    bqt_d = nc.dram_tensor("bqt", [P, NG], F32, kind="ExternalInput")
    if has_kv_bias:
        bk_d = nc.dram_tensor("bk", [P, HID], F32, kind="ExternalInput")
        bv_d = nc.dram_tensor("bv", [P, HID], F32, kind="ExternalInput")
    out_d = nc.dram_tensor("out", [S, HID], F32, kind="ExternalOutput")

    with tile.TileContext(nc) as tc:
        with (
            tc.tile_pool(name="persist", bufs=1) as pp,
            tc.tile_pool(name="work", bufs=3) as wp,
            tc.tile_pool(name="outp", bufs=3) as op,
        ):
            # ---- constants (gpsimd: all are Q-phase-only, keep the sync
            # HWDGE lane free for the critical K/V loads) ------------------
            e8_sb = pp.tile([P, KCH * H], F8, tag="e8")
            i16_sb = pp.tile([16, 16], F32, tag="i16")
            vs2_sb = pp.tile([P, 1], F32, tag="vs2")
            bqt_sb = pp.tile([P, NG], F32, tag="bqt")
            eps_sb = pp.tile([P, 1], F32, tag="eps")
            nc.vector.memset(eps_sb[:], EPS)
            if has_kv_bias:
                bk_sb = pp.tile([P, HID], F32, tag="bk")
                nc.gpsimd.dma_start(bk_sb[:], bk_d[:])
                bv_sb = pp.tile([P, HID], F32, tag="bv")
                nc.gpsimd.dma_start(bv_sb[:], bv_d[:])

            # ---- big SBUF tensors + staged loads ------------------------
            w8k_sb = pp.tile([P, KCH * HID], F8, tag="w8k")
            w8v_sb = pp.tile([P, KCH * HID], F8, tag="w8v")
            ht8_sb = pp.tile([P, KCH * S], F8, tag="ht8")
            wq_sb = pp.tile([P, KCH * HID], BF16, tag="wq")
            hta = pp.tile([P, NG * S], BF16, tag="hta")
            # SBUF layouts: w8 [p, (nj, r, o512)]; ht8 [p, (b, r, s512)];
            # wq [p, (kk, o)]; hta [p, (b, g, s512)]  (b = 512-seq block)
            w8k4 = w8k_sb[:].rearrange("p (nj r o) -> p nj r o", r=KCH, o=512)
            w8v4 = w8v_sb[:].rearrange("p (nj r o) -> p nj r o", r=KCH, o=512)
            ht84 = ht8_sb[:].rearrange("p (b r s) -> p b r s", r=KCH, s=512)
            wq3 = wq_sb[:].rearrange("p (r o) -> p r o", o=HID)
            ht4 = hta[:].rearrange("p (b g s) -> p b g s", g=NG, s=512)
            e83 = e8_sb[:].rearrange("p (r h) -> p r h", h=H)

            # Loads split over three ordered initiator lanes so the per-call
            # completion latencies of the critical K/V tensors overlap;
            # within each lane, priority order (FIFO per hw queue).
            HKB = KCH * 512
            nc.sync.dma_start(ht8_sb[:, 0:HKB], ht8_d[:, 0:HKB])
            nc.sync.dma_start(w8k_sb[:, 0:HKB], w8k_d[:, 0:HKB])
            nc.sync.dma_start(w8k_sb[:, HKB:], w8k_d[:, HKB:])
            nc.sync.dma_start(ht8_sb[:, HKB:], ht8_d[:, HKB:])
            nc.sync.dma_start(hta[:], ht_d[:])
            nc.scalar.dma_start(w8v_sb[:], w8v_d[:])
            nc.scalar.dma_start(wq_sb[:], wq_d[:])
            nc.gpsimd.dma_start(e8_sb[:], e8_d[:])
            nc.gpsimd.dma_start(i16_sb[:], i16_d[:])
            nc.gpsimd.dma_start(vs2_sb[:], vs2_d[:])
            nc.gpsimd.dma_start(bqt_sb[:], bqt_d[:])
            bqt4_sb = pp.tile([P, NG], F32, tag="bqt4")
            nc.vector.tensor_scalar_mul(bqt4_sb[:], bqt_sb[:], QSQ_SCALE)

            # ---- persistent results -------------------------------------
            qt = [pp.tile([P, S], BF16, name=f"qt{g}", tag=f"qt{g}")
                  for g in range(NG)]
            sst = pp.tile([16, S], F32, tag="sst")
            rs_all = pp.tile([P, NSC * H], F32, tag="rsall")
            rs_q = [rs_all[:, sc * H : (sc + 1) * H] for sc in range(NSC)]
            kv_sb = pp.tile([P, H * DH], BF16, tag="kvsb")
            nc.vector.memset(kv_sb[:], 0.0)

            # ---- K/V projections (fp8 DoubleRow) + kv accumulation ------
            with (
                tc.tile_pool(name="kprj", bufs=4, space="PSUM") as kvpk,
                tc.tile_pool(name="vprj", bufs=3, space="PSUM") as kvpv,
                tc.tile_pool(name="kvacc", bufs=1, space="PSUM") as kva,
            ):
                # even heads accumulate on partitions 0-63, odd on 64-127:
                # col-group tiling lets even/odd matmuls run concurrently
                kv_ps = kva.tile([P, H * 32], F32, name="kvacc", tag="kvacc")
                kn_t = {}

                def _emit_kv(esc):
                    ekn, evs = kn_t.pop(esc)
                    for hh in range(H):
                        po = (hh % 2) * DH
                        co = (hh // 2) * DH
                        nc.tensor.matmul(
                            kv_ps[po : po + DH, co : co + DH],
                            ekn[:, hh * DH : (hh + 1) * DH],
                            evs[:, hh * DH : (hh + 1) * DH],
                            start=(esc == 0 and hh < 2),
                            stop=(esc == NSC - 1 and hh >= H - 2),
                        )

                for sc in range(NSC):
                    k_ps = [kvpk.tile([P, 512], F32, name=f"kp{sc}_{i}", tag="kp")
                            for i in range(2)]
                    v_ps = [kvpv.tile([P, 512], F32, name=f"vp{sc}_{i}", tag="vp")
                            for i in range(2)]
                    scb, sco = sc // 4, (sc % 4) * P
                    for kk2 in range(KK2):
                        lhs = ht84[:, scb, 2 * kk2 : 2 * kk2 + 2,
                                   sco : sco + P]
                        for nj in range(2):
                            nc.tensor.matmul(
                                k_ps[nj][:],
                                lhs,
                                w8k4[:, nj, 2 * kk2 : 2 * kk2 + 2, :],
                                start=(kk2 == 0),
                                stop=(kk2 == KK2 - 1),
                                perf_mode=DR,
                            )
                        for nj in range(2):
                            nc.tensor.matmul(
                                v_ps[nj][:],
                                lhs,
                                w8v4[:, nj, 2 * kk2 : 2 * kk2 + 2, :],
                                start=(kk2 == 0),
                                stop=(kk2 == KK2 - 1),
                                perf_mode=DR,
                            )
                    if has_kv_bias:
                        for nj in range(2):
                            nc.vector.tensor_add(
                                k_ps[nj][:], k_ps[nj][:],
                                bk_sb[:, nj * 512 : (nj + 1) * 512],
                            )
                            nc.vector.tensor_add(
                                v_ps[nj][:], v_ps[nj][:],
                                bv_sb[:, nj * 512 : (nj + 1) * 512],
                            )

                    # row sumsq of k per head -> rs_k -> k_n; v -> bf16
                    # (chain split per 512-col half for earlier pipelining)
                    k_n = wp.tile([P, HID], BF16, name=f"kn{sc}", tag="kn",
                                  bufs=6)
                    v_sb = wp.tile([P, HID], BF16, name=f"vsb{sc}", tag="vsb",
                                  bufs=6)
                    for nj in range(2):
                        sl = slice(nj * 512, (nj + 1) * 512)
                        hsl = slice(nj * 8, (nj + 1) * 8)
                        ksq = wp.tile([P, 512], BF16, tag="ksq")
                        ssk = wp.tile([P, 8], F32, tag="ssk")
                        sqk = wp.tile([P, 8], F32, tag="sqk")
                        rsk = wp.tile([P, 8], F32, tag="rsk")
                        nc.scalar.activation(ksq[:], k_ps[nj][:], AF.Square)
                        if nj == 0:
                            nc.scalar.copy(v_sb[:, sl], v_ps[nj][:])
                        else:
                            nc.vector.tensor_copy(v_sb[:, sl], v_ps[nj][:])
                        nc.vector.tensor_reduce(
                            ssk[:],
                            ksq[:].rearrange("p (h d) -> p h d", d=DH),
                            axis=mybir.AxisListType.X,
                            op=mybir.AluOpType.add,
                        )
                        nc.scalar.activation(
                            sqk[:], ssk[:], AF.Sqrt, bias=eps_sb[:, 0:1]
                        )
                        nc.vector.reciprocal(rsk[:], sqk[:])
                        nc.vector.tensor_mul(
                            k_n[:].rearrange("p (h d) -> p h d", d=DH)[
                                :, hsl, :
                            ],
                            k_ps[nj][:].rearrange("p (h d) -> p h d", d=DH),
                            rsk[:, :, None].broadcast_to([P, 8, DH]),
                        )
                    kn_t[sc] = (k_n, v_sb)
                    # kv accumulation, deferred 1 chunk so the norm chain
                    # overlaps kv matmuls instead of stalling the PE
                    if sc >= 1:
                        _emit_kv(sc - 1)
                _emit_kv(NSC - 1)
                kvv = kv_sb[:].rearrange("p (pp two d) -> p pp two d",
                                          two=2, d=DH)
                kvp3 = kv_ps[:].rearrange("p (pp d) -> p pp d", d=DH)
                for hp2 in range(2):
                    pps = slice(hp2 * 4, (hp2 + 1) * 4)
                    nc.vector.tensor_copy(kvv[0:DH, pps, 0, :],
                                          kvp3[0:DH, pps, :])
                    nc.vector.tensor_copy(kvv[DH:P, pps, 1, :],
                                          kvp3[DH:P, pps, :])

            # ---- Q proj + sumsq + rs chain + ctx + store, per 512 seq ---
            with (
                tc.tile_pool(name="qpsum", bufs=2, space="PSUM") as qps,
                tc.tile_pool(name="sspsum", bufs=1, space="PSUM") as ssps,
                tc.tile_pool(name="rspsum", bufs=1, space="PSUM") as rsps,
                tc.tile_pool(name="ctxpsum", bufs=2, space="PSUM") as cps,
            ):
                def _emit_ctx_sc(sc):
                    # ctx for one seq chunk; kv_sb is block-diagonal per
                    # head pair, one K=128 matmul covers both heads
                    if True:
                        c_ps = cps.tile([P, HID], F32, tag="cp")
                        for pair in range(8):
                            nc.tensor.matmul(
                                c_ps[:, pair * P : (pair + 1) * P],
                                qt[pair][:, sc * P : (sc + 1) * P],
                                kv_sb[:, pair * P : (pair + 1) * P],
                                start=True,
                                stop=True,
                            )
                        out_t = op.tile([P, HID], F32, tag="outt")
                        if sc >= 4 * (NSJ - 1) and sc % 2 == 1:
                            # epilogue: scalar (idle after the last sqrt)
                            # takes every other chunk so the final scales
                            # run on two engines concurrently
                            for hh in range(H):
                                nc.scalar.activation(
                                    out_t[:, hh * DH : (hh + 1) * DH],
                                    c_ps[:, hh * DH : (hh + 1) * DH],
                                    AF.Copy,
                                    scale=rs_all[:, sc * H + hh :
                                                 sc * H + hh + 1],
                                )
                        else:
                            nc.vector.tensor_mul(
                                out_t[:].rearrange("p (h d) -> p h d", d=DH),
                                c_ps[:].rearrange("p (h d) -> p h d", d=DH),
                                rs_all[:, sc * H : (sc + 1) * H][
                                    :, :, None
                                ].broadcast_to([P, H, DH]),
                            )
                        nc.sync.dma_start(
                            out_d[sc * P : (sc + 1) * P, :], out_t[:]
                        )

                for j in range(NSJ):
                    jsl = slice(j * 512, (j + 1) * 512)
                    ss_ps = ssps.tile([16, 512], F32, tag="ssp")
                    qsq8 = None
                    for g in range(NG):
                        q_ps = qps.tile([P, 512], F32, tag="qp")
                        for kk in range(KCH):
                            nc.tensor.matmul(
                                q_ps[:],
                                wq3[:, kk, g * P : (g + 1) * P],
                                ht4[:, j, kk, :],
                                start=(kk == 0),
                                stop=(kk == KCH - 1),
                            )
                        # psum -> sbuf bf16 with per-partition bias add
                        nc.scalar.activation(
                            qt[g][:, jsl], q_ps[:], AF.Identity,
                            bias=bqt_sb[:, g : g + 1],
                        )
                        if g % 2 == 0:
                            qsq8 = wp.tile([P, 1024], F8, tag="qsq")
                        qsq83 = qsq8[:].rearrange("p (two f) -> p two f", f=512)
                        # (4q + 4bq)^2 = 16 q_biased^2 -> fp8, straight from
                        # psum so it doesn't wait on the Identity copy
                        nc.scalar.activation(
                            qsq83[:, g % 2, :], q_ps[:], AF.Square,
                            scale=QSQ_SCALE, bias=bqt4_sb[:, g : g + 1],
                        )
                        if g % 2 == 1:
                            gg = g // 2
                            nc.tensor.matmul(
                                ss_ps[:],
                                e83[:, 2 * gg : 2 * gg + 2, :],
                                qsq83[:, 0:2, :],
                                start=(gg == 0),
                                stop=(gg == KK2 - 1),
                                perf_mode=DR,
                            )
                            # spread the previous j's ctx through this
                            # j's projections: its DVE scales then overlap
                            # PE matmuls instead of bunching at the end
                            if j > 0:
                                _emit_ctx_sc(4 * (j - 1) + gg)
                    nc.vector.tensor_copy(sst[:, jsl], ss_ps[:])
                    # 4 transposes into one psum bank, then a single
                    # sqrt + reciprocal covering all 4 seq chunks of j
                    rs_ps = rsps.tile([P, 4 * H], F32, tag="rsp")
                    for i in range(4):
                        sc = 4 * j + i
                        nc.tensor.transpose(
                            rs_ps[:, i * H : (i + 1) * H],
                            sst[:, sc * P : (sc + 1) * P], i16_sb[:]
                        )
                    sq = wp.tile([P, 4 * H], F32, tag="sqq")
                    nc.scalar.activation(
                        sq[:], rs_ps[:], AF.Sqrt,
                        bias=eps_sb[:, 0:1], scale=vs2_sb[:, 0:1],
                    )
                    nc.vector.reciprocal(
                        rs_all[:, 4 * j * H : (4 * j + 4) * H], sq[:]
                    )
                for sc in range(4 * (NSJ - 1), 4 * NSJ):
                    _emit_ctx_sc(sc)

    nc.compile()
    return nc


_CACHE = {}


def _get_nc(has_kv_bias: bool):
    if has_kv_bias not in _CACHE:
        _CACHE[has_kv_bias] = build(has_kv_bias)
    return _CACHE[has_kv_bias]


def _prep_inputs(hidden_states, attention_mask, Wq, bq, Wk, bk, Wv, bv):
    """Host-side shard prep. Returns (in_maps, has_kv_bias)."""
    hs = np.asarray(hidden_states, dtype=np.float32)
    am = np.asarray(attention_mask)
    m = (am == 0).astype(np.float32).reshape(B, S)      # [B, S] valid mask
    counts = m.sum(axis=1)                               # [B]
    if not np.all(m == 1.0):
        hs = hs * m[:, :, None]                          # exact when biases==0

    wq = np.asarray(Wq, dtype=np.float32)
    wk = np.asarray(Wk, dtype=np.float32)
    wv = np.asarray(Wv, dtype=np.float32)
    bq_ = np.asarray(bq, dtype=np.float32)
    bk_ = np.asarray(bk, dtype=np.float32)
    bv_ = np.asarray(bv, dtype=np.float32)
    has_kv_bias = bool(np.any(bk_ != 0) or np.any(bv_ != 0))

    import ml_dtypes

    F8NP = ml_dtypes.float8_e4m3

    def w_img(w, dt):
        # [in, out] -> SBUF image [p, (nj, r, o512)]
        return np.ascontiguousarray(
            w.reshape(KCH, P, 2, 512).transpose(1, 2, 0, 3).reshape(P, -1)
        ).astype(dt)

    def h_img(ht, dt):
        # hT [hid, S] -> SBUF image [p, (b, r, s512)]
        return np.ascontiguousarray(
            ht.reshape(KCH, P, NSJ, 512).transpose(1, 2, 0, 3).reshape(P, -1)
        ).astype(dt)

    wq16 = np.ascontiguousarray(
        wq.reshape(KCH, P, HID).transpose(1, 0, 2).reshape(P, -1)
    ).astype(ml_dtypes.bfloat16)
    w8k = w_img(wk * WSCALE, F8NP)
    w8v = w_img(wv * WSCALE, F8NP)

    # e8[p, g*H + h] = 1 if hid index g*128+p belongs to head h
    o = np.arange(HID)
    e_full = (o[:, None] // DH == np.arange(H)[None, :]).astype(np.float32)
    e8 = np.ascontiguousarray(
        e_full.reshape(NG, P, H).transpose(1, 0, 2).reshape(P, NG * H)
    ).astype(F8NP)
    i16 = np.eye(16, dtype=np.float32)
    bqt = np.ascontiguousarray(bq_.reshape(NG, P).T)     # [128, 8]

    # k/v psum carries HSCALE*WSCALE = 4096x; ss carries 16x (QSQ_SCALE^2)
    pscale = HSCALE * WSCALE

    in_maps = []
    for b in range(B):
        htb = hs[b].T
        im = {
            "ht": h_img(htb, ml_dtypes.bfloat16),
            "ht8": h_img(htb * HSCALE, F8NP),
            "wq": wq16, "w8k": w8k, "w8v": w8v,
            "e8": e8, "i16": i16,
            "vs2": np.full(
                (P, 1),
                (np.float32(counts[b]) * pscale) ** 2 / QSQ_SCALE**2,
                np.float32,
            ),
            "bqt": bqt,
        }
        if has_kv_bias:
            im["bk"] = np.broadcast_to(bk_ * pscale, (P, HID)).copy()
            im["bv"] = np.broadcast_to(bv_ * pscale, (P, HID)).copy()
        in_maps.append(im)
    return in_maps, has_kv_bias


def run(inputs: dict, trace: bool = False, debug: bool = False):
    in_maps, has_kv_bias = _prep_inputs(**inputs)
    nc = _get_nc(has_kv_bias)
    res = run_bass_kernel_spmd(nc, in_maps, list(range(B)), trace=trace)
    out = np.stack([res.results[i]["out"] for i in range(B)]).astype(np.float32)
    return out, res


def kernel(**inputs) -> np.ndarray:
    out, _ = run(inputs)
    return out


# revision 44
# speedup vs baseline: 1.1916x; 1.0403x over previous
"""Trainium2 Bass kernel for nn_BertCosAttention (B=8, S=2048, HID=1024, H=16, DH=64).

Sharding: data-parallel over batch, 1 batch per NeuronCore, 8 cores, no
collectives.  All FLOPs on device:
  k,v = hT^T @ W      fp8e4 DoubleRow matmuls (2 contraction chunks/pass)
  q^T = Wq^T @ hT     bf16 (fp8 Q fails the accuracy budget)
  sumsq_q = E^T @ q^2 fp8 DoubleRow E-matmul -> rs_q = 1/sqrt(vs2*ss + eps)
  kv[h] = k_n[h]^T @ v[h]   bf16, accumulated over s-chunks
  ctx   = q^T^T @ kv_blockdiag, rows scaled by rs_q (DVE)
Host pre-scales the fp8 operands into e4m3 normal range (h*16, W*256); the
resulting 4096x on the k/v psum vanishes in the k l2-norm and is folded into
rs_q via vs2 = (count*4096)^2/16 (the /16 absorbs the Square-activation
scale=4 used when quantizing q^2 to fp8).
Phase order: K/V+kv first, then per 512-seq chunk: Q proj -> rs chain -> ctx
-> store, so output DMA drains during compute instead of in a tail.
"""

import numpy as np

import concourse.bacc as bacc
import concourse.mybir as mybir
from concourse import tile
from concourse.bass_utils import run_bass_kernel_spmd

B, S, HID = 8, 2048, 1024
H, DH = 16, 64
P = 128                       # partitions
NG = HID // P                 # 8 column/row groups of 128
NSC = S // P                  # 16 seq chunks of 128
NSJ = S // 512                # 4 seq chunks of 512
KCH = HID // P                # 8 contraction chunks
KK2 = KCH // 2                # 4 double-contraction chunks (fp8 DoubleRow)
EPS = 1e-24
HSCALE = 16.0                 # h -> fp8 prescale
WSCALE = 256.0                # W -> fp8 prescale
QSQ_SCALE = 4.0               # Square-activation input scale for q^2 -> fp8

F32 = mybir.dt.float32
BF16 = mybir.dt.bfloat16
F8 = mybir.dt.float8e4
AF = mybir.ActivationFunctionType
DR = mybir.MatmulPerfMode.DoubleRow


def build(has_kv_bias: bool):
    nc = bacc.Bacc("TRN2", target_bir_lowering=False, debug=False, num_devices=8)

    # All DRAM inputs are pre-arranged host-side as exact SBUF images
    # [128, free] so each load is one big call with contiguous per-partition
    # descriptors (>=4KB) instead of 128 small row descriptors.
    ht_d = nc.dram_tensor("ht", [P, NG * S], BF16, kind="ExternalInput")
    ht8_d = nc.dram_tensor("ht8", [P, KCH * S], F8, kind="ExternalInput")
    wq_d = nc.dram_tensor("wq", [P, KCH * HID], BF16, kind="ExternalInput")
    w8k_d = nc.dram_tensor("w8k", [P, KCH * HID], F8, kind="ExternalInput")
    w8v_d = nc.dram_tensor("w8v", [P, KCH * HID], F8, kind="ExternalInput")
    e8_d = nc.dram_tensor("e8", [P, KCH * H], F8, kind="ExternalInput")
    i16_d = nc.dram_tensor("i16", [16, 16], F32, kind="ExternalInput")
    vs2_d = nc.dram_tensor("vs2", [P, 1], F32, kind="ExternalInput")
    bqt_d = nc.dram_tensor("bqt", [P, NG], F32, kind="ExternalInput")
    if has_kv_bias:
        bk_d = nc.dram_tensor("bk", [P, HID], F32, kind="ExternalInput")
        bv_d = nc.dram_tensor("bv", [P, HID], F32, kind="ExternalInput")
    out_d = nc.dram_tensor("out", [S, HID], F32, kind="ExternalOutput")

    with tile.TileContext(nc) as tc:
        with (
            tc.tile_pool(name="persist", bufs=1) as pp,
            tc.tile_pool(name="work", bufs=3) as wp,
            tc.tile_pool(name="outp", bufs=3) as op,
        ):
            # ---- constants (gpsimd: all are Q-phase-only, keep the sync
            # HWDGE lane free for the critical K/V loads) ------------------
            e8_sb = pp.tile([P, KCH * H], F8, tag="e8")
            i16_sb = pp.tile([16, 16], F32, tag="i16")
            vs2_sb = pp.tile([P, 1], F32, tag="vs2")
            bqt_sb = pp.tile([P, NG], F32, tag="bqt")
            eps_sb = pp.tile([P, 1], F32, tag="eps")
            nc.vector.memset(eps_sb[:], EPS)
            if has_kv_bias:
                bk_sb = pp.tile([P, HID], F32, tag="bk")
                nc.gpsimd.dma_start(bk_sb[:], bk_d[:])
                bv_sb = pp.tile([P, HID], F32, tag="bv")
                nc.gpsimd.dma_start(bv_sb[:], bv_d[:])

            # ---- big SBUF tensors + staged loads ------------------------
            w8k_sb = pp.tile([P, KCH * HID], F8, tag="w8k")
            w8v_sb = pp.tile([P, KCH * HID], F8, tag="w8v")
            ht8_sb = pp.tile([P, KCH * S], F8, tag="ht8")
            wq_sb = pp.tile([P, KCH * HID], BF16, tag="wq")
            hta = pp.tile([P, NG * S], BF16, tag="hta")
            # SBUF layouts: w8 [p, (nj, r, o512)]; ht8 [p, (b, r, s512)];
            # wq [p, (kk, o)]; hta [p, (b, g, s512)]  (b = 512-seq block)
            w8k4 = w8k_sb[:].rearrange("p (nj r o) -> p nj r o", r=KCH, o=512)
            w8v4 = w8v_sb[:].rearrange("p (nj r o) -> p nj r o", r=KCH, o=512)
            ht84 = ht8_sb[:].rearrange("p (b r s) -> p b r s", r=KCH, s=512)
            wq3 = wq_sb[:].rearrange("p (r o) -> p r o", o=HID)
            ht4 = hta[:].rearrange("p (b g s) -> p b g s", g=NG, s=512)
            e83 = e8_sb[:].rearrange("p (r h) -> p r h", h=H)

            # Loads split over three ordered initiator lanes so the per-call
            # completion latencies of the critical K/V tensors overlap;
            # within each lane, priority order (FIFO per hw queue).
            HKB = KCH * 512
            nc.sync.dma_start(ht8_sb[:, 0:HKB], ht8_d[:, 0:HKB])
            nc.sync.dma_start(w8k_sb[:, 0:HKB], w8k_d[:, 0:HKB])
            nc.sync.dma_start(w8k_sb[:, HKB:], w8k_d[:, HKB:])
            nc.sync.dma_start(ht8_sb[:, HKB:], ht8_d[:, HKB:])
            nc.sync.dma_start(hta[:], ht_d[:])
            nc.scalar.dma_start(w8v_sb[:], w8v_d[:])
            nc.scalar.dma_start(wq_sb[:], wq_d[:])
            nc.gpsimd.dma_start(e8_sb[:], e8_d[:])
            nc.gpsimd.dma_start(i16_sb[:], i16_d[:])
            nc.gpsimd.dma_start(vs2_sb[:], vs2_d[:])
            nc.gpsimd.dma_start(bqt_sb[:], bqt_d[:])
            bqt4_sb = pp.tile([P, NG], F32, tag="bqt4")
            nc.vector.tensor_scalar_mul(bqt4_sb[:], bqt_sb[:], QSQ_SCALE)

            # ---- persistent results -------------------------------------
            qt = [pp.tile([P, S], BF16, name=f"qt{g}", tag=f"qt{g}")
                  for g in range(NG)]
            sst = pp.tile([16, S], F32, tag="sst")
            rs_all = pp.tile([P, NSC * H], F32, tag="rsall")
            rs_q = [rs_all[:, sc * H : (sc + 1) * H] for sc in range(NSC)]
            kv_sb = pp.tile([P, H * DH], BF16, tag="kvsb")
            nc.vector.memset(kv_sb[:], 0.0)

            # ---- K/V projections (fp8 DoubleRow) + kv accumulation ------
            with (
                tc.tile_pool(name="kprj", bufs=4, space="PSUM") as kvpk,
                tc.tile_pool(name="vprj", bufs=3, space="PSUM") as kvpv,
                tc.tile_pool(name="kvacc", bufs=1, space="PSUM") as kva,
            ):
                # even heads accumulate on partitions 0-63, odd on 64-127:
                # col-group tiling lets even/odd matmuls run concurrently
                kv_ps = kva.tile([P, H * 32], F32, name="kvacc", tag="kvacc")
                kn_t = {}

                def _emit_kv(esc):
                    ekn, evs = kn_t.pop(esc)
                    for hh in range(H):
                        po = (hh % 2) * DH
                        co = (hh // 2) * DH
                        nc.tensor.matmul(
                            kv_ps[po : po + DH, co : co + DH],
                            ekn[:, hh * DH : (hh + 1) * DH],
                            evs[:, hh * DH : (hh + 1) * DH],
                            start=(esc == 0 and hh < 2),
                            stop=(esc == NSC - 1 and hh >= H - 2),
                        )

                for sc in range(NSC):
                    k_ps = [kvpk.tile([P, 512], F32, name=f"kp{sc}_{i}", tag="kp")
                            for i in range(2)]
                    v_ps = [kvpv.tile([P, 512], F32, name=f"vp{sc}_{i}", tag="vp")
                            for i in range(2)]
                    scb, sco = sc // 4, (sc % 4) * P
                    for kk2 in range(KK2):
                        lhs = ht84[:, scb, 2 * kk2 : 2 * kk2 + 2,
                                   sco : sco + P]
                        for nj in range(2):
                            nc.tensor.matmul(
                                k_ps[nj][:],
                                lhs,
                                w8k4[:, nj, 2 * kk2 : 2 * kk2 + 2, :],
                                start=(kk2 == 0),
                                stop=(kk2 == KK2 - 1),
                                perf_mode=DR,
                            )
                        for nj in range(2):
                            nc.tensor.matmul(
                                v_ps[nj][:],
                                lhs,
                                w8v4[:, nj, 2 * kk2 : 2 * kk2 + 2, :],
                                start=(kk2 == 0),
                                stop=(kk2 == KK2 - 1),
                                perf_mode=DR,
                            )
                    if has_kv_bias:
                        for nj in range(2):
                            nc.vector.tensor_add(
                                k_ps[nj][:], k_ps[nj][:],
                                bk_sb[:, nj * 512 : (nj + 1) * 512],
                            )
                            nc.vector.tensor_add(
                                v_ps[nj][:], v_ps[nj][:],
                                bv_sb[:, nj * 512 : (nj + 1) * 512],
                            )

                    # row sumsq of k per head -> rs_k -> k_n; v -> bf16
                    # (chain split per 512-col half for earlier pipelining)
                    k_n = wp.tile([P, HID], BF16, name=f"kn{sc}", tag="kn",
                                  bufs=6)
                    v_sb = wp.tile([P, HID], BF16, name=f"vsb{sc}", tag="vsb",
                                  bufs=6)
                    for nj in range(2):
                        sl = slice(nj * 512, (nj + 1) * 512)
                        hsl = slice(nj * 8, (nj + 1) * 8)
                        ksq = wp.tile([P, 512], BF16, tag="ksq")
                        ssk = wp.tile([P, 8], F32, tag="ssk")
                        sqk = wp.tile([P, 8], F32, tag="sqk")
                        rsk = wp.tile([P, 8], F32, tag="rsk")
                        nc.scalar.activation(ksq[:], k_ps[nj][:], AF.Square)
                        if nj == 0:
                            nc.scalar.copy(v_sb[:, sl], v_ps[nj][:])
                        else:
                            nc.vector.tensor_copy(v_sb[:, sl], v_ps[nj][:])
                        nc.vector.tensor_reduce(
                            ssk[:],
                            ksq[:].rearrange("p (h d) -> p h d", d=DH),
                            axis=mybir.AxisListType.X,
                            op=mybir.AluOpType.add,
                        )
                        nc.scalar.activation(
                            sqk[:], ssk[:], AF.Sqrt, bias=eps_sb[:, 0:1]
                        )
                        nc.vector.reciprocal(rsk[:], sqk[:])
                        nc.vector.tensor_mul(
                            k_n[:].rearrange("p (h d) -> p h d", d=DH)[
                                :, hsl, :
                            ],
                            k_ps[nj][:].rearrange("p (h d) -> p h d", d=DH),
                            rsk[:, :, None].broadcast_to([P, 8, DH]),
                        )
                    kn_t[sc] = (k_n, v_sb)
                    # kv accumulation, deferred 2 chunks so the norm chain
                    # overlaps kv matmuls instead of stalling the PE
                    if sc >= 2:
                        _emit_kv(sc - 2)
                for sc in range(NSC - 2, NSC):
                    _emit_kv(sc)
                kvv = kv_sb[:].rearrange("p (pp two d) -> p pp two d",
                                          two=2, d=DH)
                kvp3 = kv_ps[:].rearrange("p (pp d) -> p pp d", d=DH)
                for hp2 in range(2):
                    pps = slice(hp2 * 4, (hp2 + 1) * 4)
                    nc.vector.tensor_copy(kvv[0:DH, pps, 0, :],
                                          kvp3[0:DH, pps, :])
                    nc.vector.tensor_copy(kvv[DH:P, pps, 1, :],
                                          kvp3[DH:P, pps, :])

            # ---- Q proj + sumsq + rs chain + ctx + store, per 512 seq ---
            with (
                tc.tile_pool(name="qpsum", bufs=2, space="PSUM") as qps,
                tc.tile_pool(name="sspsum", bufs=1, space="PSUM") as ssps,
                tc.tile_pool(name="rspsum", bufs=1, space="PSUM") as rsps,
                tc.tile_pool(name="ctxpsum", bufs=2, space="PSUM") as cps,
            ):
                def _emit_ctx_sc(sc):
                    # ctx for one seq chunk; kv_sb is block-diagonal per
                    # head pair, one K=128 matmul covers both heads
                    if True:
                        c_ps = cps.tile([P, HID], F32, tag="cp")
                        for pair in range(8):
                            nc.tensor.matmul(
                                c_ps[:, pair * P : (pair + 1) * P],
                                qt[pair][:, sc * P : (sc + 1) * P],
                                kv_sb[:, pair * P : (pair + 1) * P],
                                start=True,
                                stop=True,
                            )
                        out_t = op.tile([P, HID], F32, tag="outt")
                        nc.vector.tensor_mul(
                            out_t[:].rearrange("p (h d) -> p h d", d=DH),
                            c_ps[:].rearrange("p (h d) -> p h d", d=DH),
                            rs_all[:, sc * H : (sc + 1) * H][
                                :, :, None
                            ].broadcast_to([P, H, DH]),
                        )
                        nc.sync.dma_start(
                            out_d[sc * P : (sc + 1) * P, :], out_t[:]
                        )

                for j in range(NSJ):
                    jsl = slice(j * 512, (j + 1) * 512)
                    ss_ps = ssps.tile([16, 512], F32, tag="ssp")
                    qsq8 = None
                    for g in range(NG):
                        q_ps = qps.tile([P, 512], F32, tag="qp")
                        for kk in range(KCH):
                            nc.tensor.matmul(
                                q_ps[:],
                                wq3[:, kk, g * P : (g + 1) * P],
                                ht4[:, j, kk, :],
                                start=(kk == 0),
                                stop=(kk == KCH - 1),
                            )
                        # psum -> sbuf bf16 with per-partition bias add
                        nc.scalar.activation(
                            qt[g][:, jsl], q_ps[:], AF.Identity,
                            bias=bqt_sb[:, g : g + 1],
                        )
                        if g % 2 == 0:
                            qsq8 = wp.tile([P, 1024], F8, tag="qsq")
                        qsq83 = qsq8[:].rearrange("p (two f) -> p two f", f=512)
                        # (4q + 4bq)^2 = 16 q_biased^2 -> fp8, straight from
                        # psum so it doesn't wait on the Identity copy
                        nc.scalar.activation(
                            qsq83[:, g % 2, :], q_ps[:], AF.Square,
                            scale=QSQ_SCALE, bias=bqt4_sb[:, g : g + 1],
                        )
                        if g % 2 == 1:
                            gg = g // 2
                            nc.tensor.matmul(
                                ss_ps[:],
                                e83[:, 2 * gg : 2 * gg + 2, :],
                                qsq83[:, 0:2, :],
                                start=(gg == 0),
                                stop=(gg == KK2 - 1),
                                perf_mode=DR,
                            )
                            # spread the previous j's ctx through this
                            # j's projections: its DVE scales then overlap
                            # PE matmuls instead of bunching at the end
                            if j > 0:
                                _emit_ctx_sc(4 * (j - 1) + gg)
                    nc.vector.tensor_copy(sst[:, jsl], ss_ps[:])
                    # 4 transposes into one psum bank, then a single
                    # sqrt + reciprocal covering all 4 seq chunks of j
                    rs_ps = rsps.tile([P, 4 * H], F32, tag="rsp")
                    for i in range(4):
                        sc = 4 * j + i
                        nc.tensor.transpose(
                            rs_ps[:, i * H : (i + 1) * H],
                            sst[:, sc * P : (sc + 1) * P], i16_sb[:]
                        )
                    sq = wp.tile([P, 4 * H], F32, tag="sqq")
                    nc.scalar.activation(
                        sq[:], rs_ps[:], AF.Sqrt,
                        bias=eps_sb[:, 0:1], scale=vs2_sb[:, 0:1],
                    )
                    nc.vector.reciprocal(
                        rs_all[:, 4 * j * H : (4 * j + 4) * H], sq[:]
                    )
                for sc in range(4 * (NSJ - 1), 4 * NSJ):
                    _emit_ctx_sc(sc)

    nc.compile()
    return nc


_CACHE = {}


def _get_nc(has_kv_bias: bool):
    if has_kv_bias not in _CACHE:
        _CACHE[has_kv_bias] = build(has_kv_bias)
    return _CACHE[has_kv_bias]


def _prep_inputs(hidden_states, attention_mask, Wq, bq, Wk, bk, Wv, bv):
    """Host-side shard prep. Returns (in_maps, has_kv_bias)."""
    hs = np.asarray(hidden_states, dtype=np.float32)
    am = np.asarray(attention_mask)
    m = (am == 0).astype(np.float32).reshape(B, S)      # [B, S] valid mask
    counts = m.sum(axis=1)                               # [B]
    if not np.all(m == 1.0):
        hs = hs * m[:, :, None]                          # exact when biases==0

    wq = np.asarray(Wq, dtype=np.float32)
    wk = np.asarray(Wk, dtype=np.float32)
    wv = np.asarray(Wv, dtype=np.float32)
    bq_ = np.asarray(bq, dtype=np.float32)
    bk_ = np.asarray(bk, dtype=np.float32)
    bv_ = np.asarray(bv, dtype=np.float32)
    has_kv_bias = bool(np.any(bk_ != 0) or np.any(bv_ != 0))

    import ml_dtypes

    F8NP = ml_dtypes.float8_e4m3

    def w_img(w, dt):
        # [in, out] -> SBUF image [p, (nj, r, o512)]
        return np.ascontiguousarray(
            w.reshape(KCH, P, 2, 512).transpose(1, 2, 0, 3).reshape(P, -1)
        ).astype(dt)

    def h_img(ht, dt):
        # hT [hid, S] -> SBUF image [p, (b, r, s512)]
        return np.ascontiguousarray(
            ht.reshape(KCH, P, NSJ, 512).transpose(1, 2, 0, 3).reshape(P, -1)
        ).astype(dt)

    wq16 = np.ascontiguousarray(
        wq.reshape(KCH, P, HID).transpose(1, 0, 2).reshape(P, -1)
    ).astype(ml_dtypes.bfloat16)
    w8k = w_img(wk * WSCALE, F8NP)
    w8v = w_img(wv * WSCALE, F8NP)

    # e8[p, g*H + h] = 1 if hid index g*128+p belongs to head h
    o = np.arange(HID)
    e_full = (o[:, None] // DH == np.arange(H)[None, :]).astype(np.float32)
    e8 = np.ascontiguousarray(
        e_full.reshape(NG, P, H).transpose(1, 0, 2).reshape(P, NG * H)
    ).astype(F8NP)
    i16 = np.eye(16, dtype=np.float32)
    bqt = np.ascontiguousarray(bq_.reshape(NG, P).T)     # [128, 8]

    # k/v psum carries HSCALE*WSCALE = 4096x; ss carries 16x (QSQ_SCALE^2)
    pscale = HSCALE * WSCALE

    in_maps = []
    for b in range(B):
        htb = hs[b].T
        im = {
            "ht": h_img(htb, ml_dtypes.bfloat16),
            "ht8": h_img(htb * HSCALE, F8NP),
            "wq": wq16, "w8k": w8k, "w8v": w8v,
            "e8": e8, "i16": i16,
            "vs2": np.full(
                (P, 1),
                (np.float32(counts[b]) * pscale) ** 2 / QSQ_SCALE**2,
                np.float32,
            ),
            "bqt": bqt,
        }
        if has_kv_bias:
            im["bk"] = np.broadcast_to(bk_ * pscale, (P, HID)).copy()
            im["bv"] = np.broadcast_to(bv_ * pscale, (P, HID)).copy()
        in_maps.append(im)
    return in_maps, has_kv_bias


def run(inputs: dict, trace: bool = False, debug: bool = False):
    in_maps, has_kv_bias = _prep_inputs(**inputs)
    nc = _get_nc(has_kv_bias)
    res = run_bass_kernel_spmd(nc, in_maps, list(range(B)), trace=trace)
    out = np.stack([res.results[i]["out"] for i in range(B)]).astype(np.float32)
    return out, res


def kernel(**inputs) -> np.ndarray:
    out, _ = run(inputs)
    return out


# revision 46
# speedup vs baseline: 1.2335x; 1.0352x over previous
"""Trainium2 Bass kernel for nn_BertCosAttention (B=8, S=2048, HID=1024, H=16, DH=64).

Sharding: data-parallel over batch, 1 batch per NeuronCore, 8 cores, no
collectives.  All FLOPs on device:
  k,v = hT^T @ W      fp8e4 DoubleRow matmuls (2 contraction chunks/pass)
  q^T = Wq^T @ hT     bf16 (fp8 Q fails the accuracy budget)
  sumsq_q = E^T @ q^2 fp8 DoubleRow E-matmul -> rs_q = 1/sqrt(vs2*ss + eps)
  kv[h] = k_n[h]^T @ v[h]   bf16, accumulated over s-chunks
  ctx   = q^T^T @ kv_blockdiag, rows scaled by rs_q (DVE)
Host pre-scales the fp8 operands into e4m3 normal range (h*16, W*256); the
resulting 4096x on the k/v psum vanishes in the k l2-norm and is folded into
rs_q via vs2 = (count*4096)^2/16 (the /16 absorbs the Square-activation
scale=4 used when quantizing q^2 to fp8).
Phase order: K/V+kv first, then per 512-seq chunk: Q proj -> rs chain -> ctx
-> store, so output DMA drains during compute instead of in a tail.
"""

import numpy as np

import concourse.bacc as bacc
import concourse.mybir as mybir
from concourse import tile
from concourse.bass_utils import run_bass_kernel_spmd

B, S, HID = 8, 2048, 1024
H, DH = 16, 64
P = 128                       # partitions
NG = HID // P                 # 8 column/row groups of 128
NSC = S // P                  # 16 seq chunks of 128
NSJ = S // 512                # 4 seq chunks of 512
KCH = HID // P                # 8 contraction chunks
KK2 = KCH // 2                # 4 double-contraction chunks (fp8 DoubleRow)
EPS = 1e-24
HSCALE = 16.0                 # h -> fp8 prescale
WSCALE = 256.0                # W -> fp8 prescale
QSQ_SCALE = 4.0               # Square-activation input scale for q^2 -> fp8

F32 = mybir.dt.float32
BF16 = mybir.dt.bfloat16
F8 = mybir.dt.float8e4
AF = mybir.ActivationFunctionType
DR = mybir.MatmulPerfMode.DoubleRow


def build(has_kv_bias: bool):
    nc = bacc.Bacc("TRN2", target_bir_lowering=False, debug=False, num_devices=8)

    # All DRAM inputs are pre-arranged host-side as exact SBUF images
    # [128, free] so each load is one big call with contiguous per-partition
    # descriptors (>=4KB) instead of 128 small row descriptors.
    ht_d = nc.dram_tensor("ht", [P, NG * S], BF16, kind="ExternalInput")
    ht8_d = nc.dram_tensor("ht8", [P, KCH * S], F8, kind="ExternalInput")
    wq_d = nc.dram_tensor("wq", [P, KCH * HID], BF16, kind="ExternalInput")
    w8k_d = nc.dram_tensor("w8k", [P, KCH * HID], F8, kind="ExternalInput")
    w8v_d = nc.dram_tensor("w8v", [P, KCH * HID], F8, kind="ExternalInput")
    e8_d = nc.dram_tensor("e8", [P, KCH * H], F8, kind="ExternalInput")
    i16_d = nc.dram_tensor("i16", [16, 16], F32, kind="ExternalInput")
    vs2_d = nc.dram_tensor("vs2", [P, 1], F32, kind="ExternalInput")
    bqt_d = nc.dram_tensor("bqt", [P, NG], F32, kind="ExternalInput")
    if has_kv_bias:
        bk_d = nc.dram_tensor("bk", [P, HID], F32, kind="ExternalInput")
        bv_d = nc.dram_tensor("bv", [P, HID], F32, kind="ExternalInput")
    out_d = nc.dram_tensor("out", [S, HID], F32, kind="ExternalOutput")

    with tile.TileContext(nc) as tc:
        with (
            tc.tile_pool(name="persist", bufs=1) as pp,
            tc.tile_pool(name="work", bufs=3) as wp,
            tc.tile_pool(name="outp", bufs=3) as op,
        ):
            # ---- constants (gpsimd: all are Q-phase-only, keep the sync
            # HWDGE lane free for the critical K/V loads) ------------------
            e8_sb = pp.tile([P, KCH * H], F8, tag="e8")
            i16_sb = pp.tile([16, 16], F32, tag="i16")
            vs2_sb = pp.tile([P, 1], F32, tag="vs2")
            bqt_sb = pp.tile([P, NG], F32, tag="bqt")
            eps_sb = pp.tile([P, 1], F32, tag="eps")
            nc.vector.memset(eps_sb[:], EPS)
            if has_kv_bias:
                bk_sb = pp.tile([P, HID], F32, tag="bk")
                nc.gpsimd.dma_start(bk_sb[:], bk_d[:])
                bv_sb = pp.tile([P, HID], F32, tag="bv")
                nc.gpsimd.dma_start(bv_sb[:], bv_d[:])

            # ---- big SBUF tensors + staged loads ------------------------
            w8k_sb = pp.tile([P, KCH * HID], F8, tag="w8k")
            w8v_sb = pp.tile([P, KCH * HID], F8, tag="w8v")
            ht8_sb = pp.tile([P, KCH * S], F8, tag="ht8")
            wq_sb = pp.tile([P, KCH * HID], BF16, tag="wq")
            hta = pp.tile([P, NG * S], BF16, tag="hta")
            # SBUF layouts: w8 [p, (nj, r, o512)]; ht8 [p, (b, r, s512)];
            # wq [p, (kk, o)]; hta [p, (b, g, s512)]  (b = 512-seq block)
            w8k4 = w8k_sb[:].rearrange("p (nj r o) -> p nj r o", r=KCH, o=512)
            w8v4 = w8v_sb[:].rearrange("p (nj r o) -> p nj r o", r=KCH, o=512)
            ht84 = ht8_sb[:].rearrange("p (b r s) -> p b r s", r=KCH, s=512)
            wq3 = wq_sb[:].rearrange("p (r o) -> p r o", o=HID)
            ht4 = hta[:].rearrange("p (b g s) -> p b g s", g=NG, s=512)
            e83 = e8_sb[:].rearrange("p (r h) -> p r h", h=H)

            # Loads split over three ordered initiator lanes so the per-call
            # completion latencies of the critical K/V tensors overlap;
            # within each lane, priority order (FIFO per hw queue).
            HKB = KCH * 512
            nc.sync.dma_start(ht8_sb[:, 0:HKB], ht8_d[:, 0:HKB])
            nc.sync.dma_start(w8k_sb[:], w8k_d[:])
            nc.sync.dma_start(w8v_sb[:], w8v_d[:])
            nc.sync.dma_start(ht8_sb[:, HKB:], ht8_d[:, HKB:])
            nc.sync.dma_start(wq_sb[:], wq_d[:])
            nc.sync.dma_start(hta[:], ht_d[:])
            nc.gpsimd.dma_start(e8_sb[:], e8_d[:])
            nc.gpsimd.dma_start(i16_sb[:], i16_d[:])
            nc.gpsimd.dma_start(vs2_sb[:], vs2_d[:])
            nc.gpsimd.dma_start(bqt_sb[:], bqt_d[:])
            bqt4_sb = pp.tile([P, NG], F32, tag="bqt4")
            nc.vector.tensor_scalar_mul(bqt4_sb[:], bqt_sb[:], QSQ_SCALE)

            # ---- persistent results -------------------------------------
            qt = [pp.tile([P, S], BF16, name=f"qt{g}", tag=f"qt{g}")
                  for g in range(NG)]
            sst = pp.tile([16, S], F32, tag="sst")
            rs_all = pp.tile([P, NSC * H], F32, tag="rsall")
            rs_q = [rs_all[:, sc * H : (sc + 1) * H] for sc in range(NSC)]
            kv_sb = pp.tile([P, H * DH], BF16, tag="kvsb")
            nc.vector.memset(kv_sb[:], 0.0)

            # ---- K/V projections (fp8 DoubleRow) + kv accumulation ------
            with (
                tc.tile_pool(name="kprj", bufs=4, space="PSUM") as kvpk,
                tc.tile_pool(name="vprj", bufs=3, space="PSUM") as kvpv,
                tc.tile_pool(name="kvacc", bufs=1, space="PSUM") as kva,
            ):
                # even heads accumulate on partitions 0-63, odd on 64-127:
                # col-group tiling lets even/odd matmuls run concurrently
                kv_ps = kva.tile([P, H * 32], F32, name="kvacc", tag="kvacc")
                kn_t = {}

                def _emit_kv(esc):
                    ekn, evs = kn_t.pop(esc)
                    for hh in range(H):
                        po = (hh % 2) * DH
                        co = (hh // 2) * DH
                        nc.tensor.matmul(
                            kv_ps[po : po + DH, co : co + DH],
                            ekn[:, hh * DH : (hh + 1) * DH],
                            evs[:, hh * DH : (hh + 1) * DH],
                            start=(esc == 0 and hh < 2),
                            stop=(esc == NSC - 1 and hh >= H - 2),
                        )

                for sc in range(NSC):
                    k_ps = [kvpk.tile([P, 512], F32, name=f"kp{sc}_{i}", tag="kp")
                            for i in range(2)]
                    v_ps = [kvpv.tile([P, 512], F32, name=f"vp{sc}_{i}", tag="vp")
                            for i in range(2)]
                    scb, sco = sc // 4, (sc % 4) * P
                    if sc < 2:
                        # first chunks: all K matmuls before V, so the PE
                        # starts as soon as w8k lands (w8v still in flight)
                        for ps, w84 in ((k_ps, w8k4), (v_ps, w8v4)):
                            for kk2 in range(KK2):
                                lhs = ht84[:, scb, 2 * kk2 : 2 * kk2 + 2,
                                           sco : sco + P]
                                for nj in range(2):
                                    nc.tensor.matmul(
                                        ps[nj][:],
                                        lhs,
                                        w84[:, nj, 2 * kk2 : 2 * kk2 + 2, :],
                                        start=(kk2 == 0),
                                        stop=(kk2 == KK2 - 1),
                                        perf_mode=DR,
                                    )
                    else:
                        for kk2 in range(KK2):
                            lhs = ht84[:, scb, 2 * kk2 : 2 * kk2 + 2,
                                       sco : sco + P]
                            for nj in range(2):
                                nc.tensor.matmul(
                                    k_ps[nj][:],
                                    lhs,
                                    w8k4[:, nj, 2 * kk2 : 2 * kk2 + 2, :],
                                    start=(kk2 == 0),
                                    stop=(kk2 == KK2 - 1),
                                    perf_mode=DR,
                                )
                            for nj in range(2):
                                nc.tensor.matmul(
                                    v_ps[nj][:],
                                    lhs,
                                    w8v4[:, nj, 2 * kk2 : 2 * kk2 + 2, :],
                                    start=(kk2 == 0),
                                    stop=(kk2 == KK2 - 1),
                                    perf_mode=DR,
                                )
                    if has_kv_bias:
                        for nj in range(2):
                            nc.vector.tensor_add(
                                k_ps[nj][:], k_ps[nj][:],
                                bk_sb[:, nj * 512 : (nj + 1) * 512],
                            )
                            nc.vector.tensor_add(
                                v_ps[nj][:], v_ps[nj][:],
                                bv_sb[:, nj * 512 : (nj + 1) * 512],
                            )

                    # row sumsq of k per head -> rs_k -> k_n; v -> bf16
                    # (chain split per 512-col half for earlier pipelining)
                    k_n = wp.tile([P, HID], BF16, name=f"kn{sc}", tag="kn",
                                  bufs=6)
                    v_sb = wp.tile([P, HID], BF16, name=f"vsb{sc}", tag="vsb",
                                  bufs=6)
                    for nj in range(2):
                        sl = slice(nj * 512, (nj + 1) * 512)
                        hsl = slice(nj * 8, (nj + 1) * 8)
                        ksq = wp.tile([P, 512], BF16, tag="ksq")
                        ssk = wp.tile([P, 8], F32, tag="ssk")
                        sqk = wp.tile([P, 8], F32, tag="sqk")
                        rsk = wp.tile([P, 8], F32, tag="rsk")
                        nc.scalar.activation(ksq[:], k_ps[nj][:], AF.Square)
                        if nj == 0:
                            nc.scalar.copy(v_sb[:, sl], v_ps[nj][:])
                        else:
                            nc.vector.tensor_copy(v_sb[:, sl], v_ps[nj][:])
                        nc.vector.tensor_reduce(
                            ssk[:],
                            ksq[:].rearrange("p (h d) -> p h d", d=DH),
                            axis=mybir.AxisListType.X,
                            op=mybir.AluOpType.add,
                        )
                        nc.scalar.activation(
                            sqk[:], ssk[:], AF.Sqrt, bias=eps_sb[:, 0:1]
                        )
                        nc.vector.reciprocal(rsk[:], sqk[:])
                        nc.vector.tensor_mul(
                            k_n[:].rearrange("p (h d) -> p h d", d=DH)[
                                :, hsl, :
                            ],
                            k_ps[nj][:].rearrange("p (h d) -> p h d", d=DH),
                            rsk[:, :, None].broadcast_to([P, 8, DH]),
                        )
                    kn_t[sc] = (k_n, v_sb)
                    # kv accumulation, deferred 2 chunks so the norm chain
                    # overlaps kv matmuls instead of stalling the PE
                    if sc >= 2:
                        _emit_kv(sc - 2)
                for sc in range(NSC - 2, NSC):
                    _emit_kv(sc)
                kvv = kv_sb[:].rearrange("p (pp two d) -> p pp two d",
                                          two=2, d=DH)
                kvp3 = kv_ps[:].rearrange("p (pp d) -> p pp d", d=DH)
                for hp2 in range(2):
                    pps = slice(hp2 * 4, (hp2 + 1) * 4)
                    nc.vector.tensor_copy(kvv[0:DH, pps, 0, :],
                                          kvp3[0:DH, pps, :])
                    nc.vector.tensor_copy(kvv[DH:P, pps, 1, :],
                                          kvp3[DH:P, pps, :])

            # ---- Q proj + sumsq + rs chain + ctx + store, per 512 seq ---
            with (
                tc.tile_pool(name="qpsum", bufs=2, space="PSUM") as qps,
                tc.tile_pool(name="sspsum", bufs=1, space="PSUM") as ssps,
                tc.tile_pool(name="rspsum", bufs=1, space="PSUM") as rsps,
                tc.tile_pool(name="ctxpsum", bufs=2, space="PSUM") as cps,
            ):
                def _emit_ctx_sc(sc):
                    # ctx for one seq chunk; kv_sb is block-diagonal per
                    # head pair, one K=128 matmul covers both heads
                    if True:
                        c_ps = cps.tile([P, HID], F32, tag="cp")
                        for pair in range(8):
                            nc.tensor.matmul(
                                c_ps[:, pair * P : (pair + 1) * P],
                                qt[pair][:, sc * P : (sc + 1) * P],
                                kv_sb[:, pair * P : (pair + 1) * P],
                                start=True,
                                stop=True,
                            )
                        out_t = op.tile([P, HID], F32, tag="outt")
                        nc.vector.tensor_mul(
                            out_t[:].rearrange("p (h d) -> p h d", d=DH),
                            c_ps[:].rearrange("p (h d) -> p h d", d=DH),
                            rs_all[:, sc * H : (sc + 1) * H][
                                :, :, None
                            ].broadcast_to([P, H, DH]),
                        )
                        nc.sync.dma_start(
                            out_d[sc * P : (sc + 1) * P, :], out_t[:]
                        )

                for j in range(NSJ):
                    jsl = slice(j * 512, (j + 1) * 512)
                    ss_ps = ssps.tile([16, 512], F32, tag="ssp")
                    qsq8 = None
                    for g in range(NG):
                        q_ps = qps.tile([P, 512], F32, tag="qp")
                        for kk in range(KCH):
                            nc.tensor.matmul(
                                q_ps[:],
                                wq3[:, kk, g * P : (g + 1) * P],
                                ht4[:, j, kk, :],
                                start=(kk == 0),
                                stop=(kk == KCH - 1),
                            )
                        # psum -> sbuf bf16 with per-partition bias add
                        nc.scalar.activation(
                            qt[g][:, jsl], q_ps[:], AF.Identity,
                            bias=bqt_sb[:, g : g + 1],
                        )
                        if g % 2 == 0:
                            qsq8 = wp.tile([P, 1024], F8, tag="qsq")
                        qsq83 = qsq8[:].rearrange("p (two f) -> p two f", f=512)
                        # (4q + 4bq)^2 = 16 q_biased^2 -> fp8, straight from
                        # psum so it doesn't wait on the Identity copy
                        nc.scalar.activation(
                            qsq83[:, g % 2, :], q_ps[:], AF.Square,
                            scale=QSQ_SCALE, bias=bqt4_sb[:, g : g + 1],
                        )
                        if g % 2 == 1:
                            gg = g // 2
                            nc.tensor.matmul(
                                ss_ps[:],
                                e83[:, 2 * gg : 2 * gg + 2, :],
                                qsq83[:, 0:2, :],
                                start=(gg == 0),
                                stop=(gg == KK2 - 1),
                                perf_mode=DR,
                            )
                            # spread the previous j's ctx through this
                            # j's projections: its DVE scales then overlap
                            # PE matmuls instead of bunching at the end
                            if j > 0:
                                _emit_ctx_sc(4 * (j - 1) + gg)
                    nc.vector.tensor_copy(sst[:, jsl], ss_ps[:])
                    # 4 transposes into one psum bank, then a single
                    # sqrt + reciprocal covering all 4 seq chunks of j
                    rs_ps = rsps.tile([P, 4 * H], F32, tag="rsp")
                    for i in range(4):
                        sc = 4 * j + i
                        nc.tensor.transpose(
                            rs_ps[:, i * H : (i + 1) * H],
                            sst[:, sc * P : (sc + 1) * P], i16_sb[:]
                        )
                    sq = wp.tile([P, 4 * H], F32, tag="sqq")
                    nc.scalar.activation(
                        sq[:], rs_ps[:], AF.Sqrt,
                        bias=eps_sb[:, 0:1], scale=vs2_sb[:, 0:1],
                    )
                    nc.vector.reciprocal(
                        rs_all[:, 4 * j * H : (4 * j + 4) * H], sq[:]
                    )
                for sc in range(4 * (NSJ - 1), 4 * NSJ):
                    _emit_ctx_sc(sc)

    nc.compile()
    return nc


_CACHE = {}


def _get_nc(has_kv_bias: bool):
    if has_kv_bias not in _CACHE:
        _CACHE[has_kv_bias] = build(has_kv_bias)
    return _CACHE[has_kv_bias]


def _prep_inputs(hidden_states, attention_mask, Wq, bq, Wk, bk, Wv, bv):
    """Host-side shard prep. Returns (in_maps, has_kv_bias)."""
    hs = np.asarray(hidden_states, dtype=np.float32)
    am = np.asarray(attention_mask)
    m = (am == 0).astype(np.float32).reshape(B, S)      # [B, S] valid mask
    counts = m.sum(axis=1)                               # [B]
    if not np.all(m == 1.0):
        hs = hs * m[:, :, None]                          # exact when biases==0

    wq = np.asarray(Wq, dtype=np.float32)
    wk = np.asarray(Wk, dtype=np.float32)
    wv = np.asarray(Wv, dtype=np.float32)
    bq_ = np.asarray(bq, dtype=np.float32)
    bk_ = np.asarray(bk, dtype=np.float32)
    bv_ = np.asarray(bv, dtype=np.float32)
    has_kv_bias = bool(np.any(bk_ != 0) or np.any(bv_ != 0))

    import ml_dtypes

    F8NP = ml_dtypes.float8_e4m3

    def w_img(w, dt):
        # [in, out] -> SBUF image [p, (nj, r, o512)]
        return np.ascontiguousarray(
            w.reshape(KCH, P, 2, 512).transpose(1, 2, 0, 3).reshape(P, -1)
        ).astype(dt)

    def h_img(ht, dt):
        # hT [hid, S] -> SBUF image [p, (b, r, s512)]
        return np.ascontiguousarray(
            ht.reshape(KCH, P, NSJ, 512).transpose(1, 2, 0, 3).reshape(P, -1)
        ).astype(dt)

    wq16 = np.ascontiguousarray(
        wq.reshape(KCH, P, HID).transpose(1, 0, 2).reshape(P, -1)
    ).astype(ml_dtypes.bfloat16)
    w8k = w_img(wk * WSCALE, F8NP)
    w8v = w_img(wv * WSCALE, F8NP)

    # e8[p, g*H + h] = 1 if hid index g*128+p belongs to head h
    o = np.arange(HID)
    e_full = (o[:, None] // DH == np.arange(H)[None, :]).astype(np.float32)
    e8 = np.ascontiguousarray(
        e_full.reshape(NG, P, H).transpose(1, 0, 2).reshape(P, NG * H)
    ).astype(F8NP)
    i16 = np.eye(16, dtype=np.float32)
    bqt = np.ascontiguousarray(bq_.reshape(NG, P).T)     # [128, 8]

    # k/v psum carries HSCALE*WSCALE = 4096x; ss carries 16x (QSQ_SCALE^2)
    pscale = HSCALE * WSCALE

    in_maps = []
    for b in range(B):
        htb = hs[b].T
        im = {
            "ht": h_img(htb, ml_dtypes.bfloat16),
            "ht8": h_img(htb * HSCALE, F8NP),
            "wq": wq16, "w8k": w8k, "w8v": w8v,
            "e8": e8, "i16": i16,
            "vs2": np.full(
                (P, 1),
                (np.float32(counts[b]) * pscale) ** 2 / QSQ_SCALE**2,
                np.float32,
            ),
            "bqt": bqt,
        }
        if has_kv_bias:
            im["bk"] = np.broadcast_to(bk_ * pscale, (P, HID)).copy()
            im["bv"] = np.broadcast_to(bv_ * pscale, (P, HID)).copy()
        in_maps.append(im)
    return in_maps, has_kv_bias


def run(inputs: dict, trace: bool = False, debug: bool = False):
    in_maps, has_kv_bias = _prep_inputs(**inputs)
    nc = _get_nc(has_kv_bias)
    res = run_bass_kernel_spmd(nc, in_maps, list(range(B)), trace=trace)
    out = np.stack([res.results[i]["out"] for i in range(B)]).astype(np.float32)
    return out, res


def kernel(**inputs) -> np.ndarray:
    out, _ = run(inputs)
    return out
